# revision 3
# baseline (speedup 1.0000x reference)
"""Event-RGB dynamic fusion module on 8 trn2 NeuronCores.

Per-pixel dynamic 3x3 depthwise kernels predicted from concat(rgb, event)
via two 1x1 convs + relu, applied to reflect-padded rgb.

Sharding: 8 shards = (batch b in 0..3) x (H half in {0,1}); each core gets
reflect-padded rgb slabs (two bf16 copies at element offsets 0/1 so every
3x3-shift view stays 4-byte aligned for DVE 2x mode), a bf16 event slab,
and replicated pre-laid-out bf16 weights. Fully data-parallel, no
collectives.

Pipeline per 4-row sub-slice (rows packed as [128] = 64ch x {top,bottom}):
  mm1 (PE, K=96 via 64+32 accum) -> ph psum, relu+b1 on ACT -> h4 bf16
  mm2 (PE, 9 taps x [K=32,M=64] row/col-group packed) -> dk psum fp32
  taps: 6x fused STT on DVE ((dk+b2)*patch straight from PSUM),
        3x ACT bias-copy to bf16 + GPSIMD mul
  reduction: 9 prods summed on the PE via identity-matmul PSUM
  accumulation (2 chains of 9 accumulating matmuls), ACT copies the
  fp32 result to SBUF, DMA out.
"""

import os
from contextlib import ExitStack

import ml_dtypes
import numpy as np

import concourse.bass as bass
import concourse.bacc as bacc
import concourse.mybir as mybir
import concourse.tile as tile
from concourse.bass_utils import run_bass_kernel_spmd

B, C, H, W = 4, 64, 256, 256
CEV, KK, MID = 32, 3, 32
NCORES = 8
SHARD_H = 128          # rows per core
HALF = 64              # rows per half (partition-packing of pixel halves)
RBLK = 16              # rows per half per block
NBLK = HALF // RBLK    # 4
WE = 260               # padded row length (even, so shifted views stay aligned)
SUBR = 4               # rows per half per mm2/apply sub-slice (=1024 px)
NSUB = RBLK // SUBR    # 4
F32 = mybir.dt.float32
BF16 = mybir.dt.bfloat16
AOP = mybir.AluOpType
RELU = mybir.ActivationFunctionType.Relu
IDENT = mybir.ActivationFunctionType.Identity
BF = ml_dtypes.bfloat16

# apply-path per tap ij: "A" = fused STT on DVE ((dk+b2)*patch with dk read
# straight from PSUM fp32); "B" = ACT copy+bias to bf16 then mul on GPSIMD.
PATHS = ["A", "B", "A", "A", "B", "A", "A", "B", "A"]
# mm2/extraction processing order: GPSIMD-fed taps first (their muls are
# the slowest, start them early); id-MM accumulation order: DVE taps first.
TAP_ORDER = [1, 4, 7, 0, 2, 3, 5, 6, 8]
IDMM_ORDER = [0, 2, 3, 5, 6, 8, 1, 4, 7]

_cache = {}


def _build():
    nc = bacc.Bacc("TRN2", target_bir_lowering=False, debug=False)
    rgbe = nc.dram_tensor("rgbe", [C, SHARD_H + 2, WE], BF16, kind="ExternalInput").ap()
    rgbo = nc.dram_tensor("rgbo", [C, SHARD_H + 2, WE], BF16, kind="ExternalInput").ap()
    ev = nc.dram_tensor("ev", [CEV, SHARD_H, W], BF16, kind="ExternalInput").ap()
    w1 = nc.dram_tensor("w1", [128, 384], BF16, kind="ExternalInput").ap()
    w2 = nc.dram_tensor("w2", [128, 384], BF16, kind="ExternalInput").ap()
    bi = nc.dram_tensor("bi", [128, 10], F32, kind="ExternalInput").ap()
    idm = nc.dram_tensor("idm", [128, 128], BF16, kind="ExternalInput").ap()
    out = nc.dram_tensor("out", [C, SHARD_H, W], F32, kind="ExternalOutput").ap()

    with tile.TileContext(nc) as tc, ExitStack() as ctx:
        _kernel(ctx, tc, rgbe, rgbo, ev, w1, w2, bi, idm, out)
    nc.compile()
    return nc


def _kernel(ctx, tc, rgbe, rgbo, ev, w1, w2, bi, idm, out):
    nc = tc.nc
    consts = ctx.enter_context(tc.tile_pool(name="consts", bufs=1))
    rgb_p = ctx.enter_context(tc.tile_pool(name="rgb", bufs=2))
    ev_p = ctx.enter_context(tc.tile_pool(name="evp", bufs=2))
    h4_p = ctx.enter_context(tc.tile_pool(name="h4", bufs=2))
    dkb_p = ctx.enter_context(tc.tile_pool(name="dkb", bufs=4))
    prod_p = ctx.enter_context(tc.tile_pool(name="prod", bufs=12))
    outt_p = ctx.enter_context(tc.tile_pool(name="outt", bufs=4))
    ph_p = ctx.enter_context(tc.tile_pool(name="psum_h", bufs=2, space="PSUM"))
    pdk_p = ctx.enter_context(tc.tile_pool(name="psum_dk", bufs=2, space="PSUM"))
    po_p = ctx.enter_context(tc.tile_pool(name="psum_o", bufs=2, space="PSUM"))

    w1t = consts.tile([128, 384], BF16)
    nc.sync.dma_start(w1t[:], w1[:])
    w2t = consts.tile([128, 384], BF16)
    nc.sync.dma_start(w2t[:], w2[:])
    bt = consts.tile([128, 10], F32)
    nc.sync.dma_start(bt[:], bi[:])
    idt = consts.tile([128, 128], BF16)
    nc.sync.dma_start(idt[:], idm[:])

    npx = SUBR * W           # pixels per half per sub-slice (1024)

    for t in range(NBLK):
        rge = rgb_p.tile([128, (RBLK + 2) * WE], BF16, tag="rge")
        nc.sync.dma_start(rge[0:64, :], rgbe[:, t * RBLK:t * RBLK + RBLK + 2, :])
        nc.sync.dma_start(
            rge[64:128, :], rgbe[:, HALF + t * RBLK:HALF + t * RBLK + RBLK + 2, :])
        rgo = rgb_p.tile([128, (RBLK + 2) * WE], BF16, tag="rgo")
        nc.sync.dma_start(rgo[0:64, :], rgbo[:, t * RBLK:t * RBLK + RBLK + 2, :])
        nc.sync.dma_start(
            rgo[64:128, :], rgbo[:, HALF + t * RBLK:HALF + t * RBLK + RBLK + 2, :])
        evt = ev_p.tile([128, RBLK * W], BF16)
        nc.sync.dma_start(evt[64:96, :], ev[:, t * RBLK:t * RBLK + RBLK, :])
        nc.sync.dma_start(
            evt[96:128, :], ev[:, HALF + t * RBLK:HALF + t * RBLK + RBLK, :])

        rgev = rge[:].rearrange("p (r w) -> p r w", w=WE)      # [128, 18, 260]
        rgov = rgo[:].rearrange("p (r w) -> p r w", w=WE)
        evv = evt[:].rearrange("p (r w) -> p r w", w=W)        # [128, 16, 256]

        for s in range(NSUB):
            # ---- mm1: h4[32q+m, px] = relu(b1 + W1 @ concat(rgb, ev)) ----
            # h4 cols: [0:1024] = half A pixels, [1024:2048] = half B.
            h4 = h4_p.tile([128, 2 * npx], BF16)
            for sl in range(2):                  # 512-px slices per half
                r0 = SUBR * s + 2 * sl
                ph = ph_p.tile([128, 512], F32, tag="ph")
                ph2 = ph_p.tile([128, 512], F32, tag="ph")
                # A-rgb (rows 0-63) and B-rgb (rows 64-127) stream
                # concurrently, then A-ev (64-95) and B-ev (96-127).
                nc.tensor.matmul(ph[:], w1t[0:64, 0:128],
                                 rgev[0:64, r0 + 1:r0 + 3, 2:258],
                                 start=True, stop=False, tile_position=(0, 0))
                nc.tensor.matmul(ph2[:], w1t[64:128, 128:256],
                                 rgev[64:128, r0 + 1:r0 + 3, 2:258],
                                 start=True, stop=False, tile_position=(64, 0))
                nc.tensor.matmul(ph[:], w1t[64:96, 0:128],
                                 evv[64:96, r0:r0 + 2, :],
                                 start=False, stop=True, tile_position=(64, 0))
                nc.tensor.matmul(ph2[:], w1t[96:128, 256:384],
                                 evv[96:128, r0:r0 + 2, :],
                                 start=False, stop=True, tile_position=(96, 0))
                nc.scalar.activation(h4[:, 512 * sl:512 * (sl + 1)], ph[:],
                                     RELU, bias=bt[:, 0:1], scale=1.0)
                nc.scalar.activation(
                    h4[:, npx + 512 * sl:npx + 512 * (sl + 1)], ph2[:],
                    RELU, bias=bt[:, 0:1], scale=1.0)

            # ---- mm2 + per-tap product ----
            prods = {}
            for ij in TAP_ORDER:
                rg, slot = ij % 4, ij // 4
                dk = pdk_p.tile([128, 1024], F32, name="dk", tag="dk")
                for hf in range(2):
                    lh = w2t[32 * rg:32 * rg + 32,
                             128 * slot + 64 * hf:128 * slot + 64 * hf + 64]
                    for nh in range(2):
                        nc.tensor.matmul(
                            dk[64 * hf:64 * hf + 64, 512 * nh:512 * nh + 512],
                            lh,
                            h4[32 * rg:32 * rg + 32,
                               npx * hf + 512 * nh:npx * hf + 512 * nh + 512],
                            start=True, stop=True,
                            tile_position=(32 * rg, 64 * hf))
                di, dj = ij // 3 - 1, ij % 3 - 1
                # patch view: dj=0 from the even-aligned slab, dj=+-1 from
                # the odd one (keeps every bf16 view 4B-aligned).
                if dj == 0:
                    src, base = rgev, 2
                else:
                    src, base = rgov, 1 + dj
                patch = src[:, SUBR * s + 1 + di:SUBR * s + 5 + di,
                            base:base + 256]
                dkv = dk[:].rearrange("p (r w) -> p r w", w=W)
                prod = prod_p.tile([128, 1024], BF16)
                prodv = prod[:].rearrange("p (r w) -> p r w", w=W)
                if PATHS[ij] == "A":
                    nc.vector.scalar_tensor_tensor(
                        prodv[:], dkv[:], bt[:, 1 + ij:2 + ij], patch[:],
                        op0=AOP.add, op1=AOP.mult)
                else:
                    dkb = dkb_p.tile([128, 1024], BF16)
                    nc.scalar.activation(dkb[:], dk[:], IDENT,
                                         bias=bt[:, 1 + ij:2 + ij], scale=1.0)
                    dkbv = dkb[:].rearrange("p (r w) -> p r w", w=W)
                    nc.gpsimd.tensor_tensor(prodv[:], dkbv[:], patch[:],
                                            op=AOP.mult)
                prods[ij] = prod

            # ---- reduction: out[:, px] = sum_ij prods[ij] on the PE via
            # identity-matmul accumulation into two psum chains.
            oc0 = po_p.tile([128, 512], F32, tag="oc")
            oc1 = po_p.tile([128, 512], F32, tag="oc")
            for k, ij in enumerate(IDMM_ORDER):
                st, sp = (k == 0), (k == len(IDMM_ORDER) - 1)
                nc.tensor.matmul(oc0[:], idt[:], prods[ij][:, 0:512],
                                 start=st, stop=sp, tile_position=(0, 0))
                nc.tensor.matmul(oc1[:], idt[:], prods[ij][:, 512:1024],
                                 start=st, stop=sp, tile_position=(0, 0))

            ot = outt_p.tile([128, 1024], F32)
            nc.scalar.copy(ot[:, 0:512], oc0[:])
            nc.scalar.copy(ot[:, 512:1024], oc1[:])

            otv = ot[:].rearrange("p (r w) -> p r w", w=W)
            ra = t * RBLK + SUBR * s
            nc.sync.dma_start(out[:, ra:ra + SUBR, :], otv[0:64, :, :])
            nc.sync.dma_start(out[:, HALF + ra:HALF + ra + SUBR, :],
                              otv[64:128, :, :])


def _prep_consts(W1, b1, W2, b2):
    W1T = np.ascontiguousarray(W1.T)                              # [96, 32]
    W1T4 = np.tile(W1T, (1, 4))                                   # [96, 128]
    w1sb = np.zeros((128, 384), np.float32)
    w1sb[0:64, 0:128] = W1T4[0:64]          # rgb A
    w1sb[64:96, 0:128] = W1T4[64:96]        # ev A
    w1sb[64:128, 128:256] = W1T4[0:64]      # rgb B
    w1sb[96:128, 256:384] = W1T4[64:96]     # ev B

    W2r = W2.reshape(C, 9, MID)
    w2sb = np.zeros((128, 384), np.float32)
    for ij in range(9):
        rg, slot = ij % 4, ij // 4
        wij = np.ascontiguousarray(W2r[:, ij, :].T)               # [32, 64]
        w2sb[32 * rg:32 * rg + 32, 128 * slot:128 * slot + 64] = wij
        w2sb[32 * rg:32 * rg + 32, 128 * slot + 64:128 * slot + 128] = wij

    bisb = np.zeros((128, 10), np.float32)
    bisb[:, 0] = np.tile(b1, 4)
    b2r = b2.reshape(C, 9)
    for ij in range(9):
        bisb[:, 1 + ij] = np.concatenate([b2r[:, ij], b2r[:, ij]])
    return w1sb.astype(BF), w2sb.astype(BF), bisb


def _shard_inputs(rgb_feature, event_feature, W1, b1, W2, b2):
    rgbp = np.pad(rgb_feature, ((0, 0), (0, 0), (1, 1), (1, 1)), mode="reflect")
    # two bf16 copies of the padded slab: pixel col c at element c+2 (even
    # view, serves dj=0) and at element c+1 (odd view, serves dj=+-1).
    rgbe = np.zeros((B, C, H + 2, WE), BF)
    rgbo = np.zeros((B, C, H + 2, WE), BF)
    rgbe[:, :, :, 1:1 + W + 2] = rgbp
    rgbo[:, :, :, 0:W + 2] = rgbp
    evb = event_feature.astype(BF)
    w1sb, w2sb, bisb = _prep_consts(W1, b1, W2, b2)
    idsb = np.eye(128, dtype=BF)
    in_maps = []
    for k in range(NCORES):
        b, r0 = k // 2, SHARD_H * (k % 2)
        in_maps.append({
            "rgbe": np.ascontiguousarray(rgbe[b, :, r0:r0 + SHARD_H + 2, :]),
            "rgbo": np.ascontiguousarray(rgbo[b, :, r0:r0 + SHARD_H + 2, :]),
            "ev": np.ascontiguousarray(evb[b, :, r0:r0 + SHARD_H, :]),
            "w1": w1sb, "w2": w2sb, "bi": bisb, "idm": idsb,
        })
    return in_maps


def _run(inputs, trace=False, **trace_kwargs):
    if "nc" not in _cache:
        _cache["nc"] = _build()
    nc = _cache["nc"]
    in_maps = _shard_inputs(
        inputs["rgb_feature"].astype(np.float32),
        inputs["event_feature"].astype(np.float32),
        inputs["W1"].astype(np.float32), inputs["b1"].astype(np.float32),
        inputs["W2"].astype(np.float32), inputs["b2"].astype(np.float32))
    res = run_bass_kernel_spmd(nc, in_maps, list(range(NCORES)),
                               trace=trace, **trace_kwargs)
    full = np.empty((B, C, H, W), np.float32)
    for k in range(NCORES):
        b, r0 = k // 2, SHARD_H * (k % 2)
        full[b, :, r0:r0 + SHARD_H, :] = res.results[k]["out"]
    return full, res


def kernel(**inputs):
    full, _ = _run(inputs, trace=False)
    return full


# revision 9
# speedup vs baseline: 1.0809x; 1.0809x over previous
"""Event-RGB dynamic fusion module on 8 trn2 NeuronCores.

Per-pixel dynamic 3x3 depthwise kernels predicted from concat(rgb, event)
via two 1x1 convs + relu, applied to reflect-padded rgb.

Sharding: 8 shards = (batch b in 0..3) x (H half in {0,1}); each core gets
reflect-padded rgb slabs (two bf16 copies at element offsets 0/1 so every
3x3-shift view stays 4-byte aligned for DVE 2x mode), a bf16 event slab,
and replicated pre-laid-out bf16 weights. Fully data-parallel, no
collectives.

Pipeline per 4-row sub-slice (rows packed as [128] = 64ch x {top,bottom}):
  mm1 (PE, K=96 via 64+32 accum) -> ph psum, relu+b1 on ACT -> h4 bf16
  mm2 (PE, 9 taps x [K=32,M=64] row/col-group packed) -> dk psum fp32
  taps: 6x fused STT on DVE ((dk+b2)*patch straight from PSUM),
        3x ACT bias-copy to bf16 + GPSIMD mul
  reduction: 9 prods summed on the PE via identity-matmul PSUM
  accumulation (2 chains of 9 accumulating matmuls), ACT copies the
  fp32 result to SBUF, DMA out.
"""

import os
from contextlib import ExitStack

import ml_dtypes
import numpy as np

import concourse.bass as bass
import concourse.bacc as bacc
import concourse.mybir as mybir
import concourse.tile as tile
from concourse.bass_utils import run_bass_kernel_spmd

B, C, H, W = 4, 64, 256, 256
CEV, KK, MID = 32, 3, 32
NCORES = 8
SHARD_H = 128          # rows per core
HALF = 64              # rows per half (partition-packing of pixel halves)
RBLK = 16              # rows per half per block
NBLK = HALF // RBLK    # 4
WE = 260               # padded row length (even, so shifted views stay aligned)
SUBR = 4               # rows per half per mm2/apply sub-slice (=1024 px)
NSUB = RBLK // SUBR    # 4
F32 = mybir.dt.float32
BF16 = mybir.dt.bfloat16
AOP = mybir.AluOpType
RELU = mybir.ActivationFunctionType.Relu
IDENT = mybir.ActivationFunctionType.Identity
BF = ml_dtypes.bfloat16

# apply-path per tap ij: "A" = fused STT on DVE ((dk+b2)*patch with dk read
# straight from PSUM fp32); "B" = ACT copy+bias to bf16 then mul on GPSIMD;
# "C" = ACT copy+bias to bf16 then mul on DVE.
PATHS = ["A", "B", "A", "A", "B", "A", "A", "B", "C"]
# mm2/extraction processing order: conv-fed taps first (their muls are the
# slowest, start them early), and adjacent taps hit distinct PE row groups.
TAP_ORDER = [1, 4, 7, 8, 0, 2, 5, 3, 6]
# prods 1,4 merge on DVE, 7,8 on GPSIMD, the two sums merge on DVE; the PE
# id-MM chain consumes taps 0,2,5,3,6 then the merged tile.
IDMM_ORDER = [0, 2, 5, 3, 6]

_cache = {}


def _build():
    nc = bacc.Bacc("TRN2", target_bir_lowering=False, debug=False)
    rgbe = nc.dram_tensor("rgbe", [C, SHARD_H + 2, WE], BF16, kind="ExternalInput").ap()
    rgbo = nc.dram_tensor("rgbo", [C, SHARD_H + 2, WE], BF16, kind="ExternalInput").ap()
    ev = nc.dram_tensor("ev", [CEV, SHARD_H, W], BF16, kind="ExternalInput").ap()
    w1 = nc.dram_tensor("w1", [128, 384], BF16, kind="ExternalInput").ap()
    w2 = nc.dram_tensor("w2", [128, 384], BF16, kind="ExternalInput").ap()
    bi = nc.dram_tensor("bi", [128, 10], F32, kind="ExternalInput").ap()
    idm = nc.dram_tensor("idm", [128, 128], BF16, kind="ExternalInput").ap()
    out = nc.dram_tensor("out", [C, SHARD_H, W], F32, kind="ExternalOutput").ap()

    with tile.TileContext(nc) as tc, ExitStack() as ctx:
        _kernel(ctx, tc, rgbe, rgbo, ev, w1, w2, bi, idm, out)
    nc.compile()
    return nc


def _kernel(ctx, tc, rgbe, rgbo, ev, w1, w2, bi, idm, out):
    nc = tc.nc
    consts = ctx.enter_context(tc.tile_pool(name="consts", bufs=1))
    rgb_p = ctx.enter_context(tc.tile_pool(name="rgb", bufs=2))
    ev_p = ctx.enter_context(tc.tile_pool(name="evp", bufs=2))
    h4_p = ctx.enter_context(tc.tile_pool(name="h4", bufs=2))
    dkb_p = ctx.enter_context(tc.tile_pool(name="dkb", bufs=4))
    prod_p = ctx.enter_context(tc.tile_pool(name="prod", bufs=16))
    outt_p = ctx.enter_context(tc.tile_pool(name="outt", bufs=4))
    ph_p = ctx.enter_context(tc.tile_pool(name="psum_h", bufs=2, space="PSUM"))
    pdk_p = ctx.enter_context(tc.tile_pool(name="psum_dk", bufs=2, space="PSUM"))
    po_p = ctx.enter_context(tc.tile_pool(name="psum_o", bufs=2, space="PSUM"))

    w1t = consts.tile([128, 384], BF16)
    nc.sync.dma_start(w1t[:], w1[:])
    w2t = consts.tile([128, 384], BF16)
    nc.sync.dma_start(w2t[:], w2[:])
    bt = consts.tile([128, 10], F32)
    nc.sync.dma_start(bt[:], bi[:])
    idt = consts.tile([128, 128], BF16)
    nc.sync.dma_start(idt[:], idm[:])

    npx = SUBR * W           # pixels per half per sub-slice (1024)

    for t in range(NBLK):
        rge = rgb_p.tile([128, (RBLK + 2) * WE], BF16, tag="rge")
        nc.sync.dma_start(rge[0:64, :], rgbe[:, t * RBLK:t * RBLK + RBLK + 2, :])
        nc.sync.dma_start(
            rge[64:128, :], rgbe[:, HALF + t * RBLK:HALF + t * RBLK + RBLK + 2, :])
        rgo = rgb_p.tile([128, (RBLK + 2) * WE], BF16, tag="rgo")
        nc.sync.dma_start(rgo[0:64, :], rgbo[:, t * RBLK:t * RBLK + RBLK + 2, :])
        nc.sync.dma_start(
            rgo[64:128, :], rgbo[:, HALF + t * RBLK:HALF + t * RBLK + RBLK + 2, :])
        evt = ev_p.tile([128, RBLK * W], BF16)
        nc.sync.dma_start(evt[64:96, :], ev[:, t * RBLK:t * RBLK + RBLK, :])
        nc.sync.dma_start(
            evt[96:128, :], ev[:, HALF + t * RBLK:HALF + t * RBLK + RBLK, :])

        rgev = rge[:].rearrange("p (r w) -> p r w", w=WE)      # [128, 18, 260]
        rgov = rgo[:].rearrange("p (r w) -> p r w", w=WE)
        evv = evt[:].rearrange("p (r w) -> p r w", w=W)        # [128, 16, 256]

        for s in range(NSUB):
            # ---- mm1: h4[32q+m, px] = relu(b1 + W1 @ concat(rgb, ev)) ----
            # h4 cols: [0:1024] = half A pixels, [1024:2048] = half B.
            h4 = h4_p.tile([128, 2 * npx], BF16)
            for sl in range(2):                  # 512-px slices per half
                r0 = SUBR * s + 2 * sl
                ph = ph_p.tile([128, 512], F32, tag="ph")
                ph2 = ph_p.tile([128, 512], F32, tag="ph")
                # A-rgb (rows 0-63) and B-rgb (rows 64-127) stream
                # concurrently, then A-ev (64-95) and B-ev (96-127).
                nc.tensor.matmul(ph[:], w1t[0:64, 0:128],
                                 rgev[0:64, r0 + 1:r0 + 3, 2:258],
                                 start=True, stop=False, tile_position=(0, 0))
                nc.tensor.matmul(ph2[:], w1t[64:128, 128:256],
                                 rgev[64:128, r0 + 1:r0 + 3, 2:258],
                                 start=True, stop=False, tile_position=(64, 0))
                nc.tensor.matmul(ph[:], w1t[64:96, 0:128],
                                 evv[64:96, r0:r0 + 2, :],
                                 start=False, stop=True, tile_position=(64, 0))
                nc.tensor.matmul(ph2[:], w1t[96:128, 256:384],
                                 evv[96:128, r0:r0 + 2, :],
                                 start=False, stop=True, tile_position=(96, 0))
                nc.scalar.activation(h4[:, 512 * sl:512 * (sl + 1)], ph[:],
                                     RELU, bias=bt[:, 0:1], scale=1.0)
                nc.scalar.activation(
                    h4[:, npx + 512 * sl:npx + 512 * (sl + 1)], ph2[:],
                    RELU, bias=bt[:, 0:1], scale=1.0)

            # ---- mm2 + per-tap product ----
            prods = {}
            for ij in TAP_ORDER:
                rg, slot = ij % 4, ij // 4
                dk = pdk_p.tile([128, 1024], F32, name="dk", tag="dk")
                for hf in range(2):
                    lh = w2t[32 * rg:32 * rg + 32,
                             128 * slot + 64 * hf:128 * slot + 64 * hf + 64]
                    for nh in range(2):
                        nc.tensor.matmul(
                            dk[64 * hf:64 * hf + 64, 512 * nh:512 * nh + 512],
                            lh,
                            h4[32 * rg:32 * rg + 32,
                               npx * hf + 512 * nh:npx * hf + 512 * nh + 512],
                            start=True, stop=True,
                            tile_position=(32 * rg, 64 * hf))
                di, dj = ij // 3 - 1, ij % 3 - 1
                # patch view: dj=0 from the even-aligned slab, dj=+-1 from
                # the odd one (keeps every bf16 view 4B-aligned).
                if dj == 0:
                    src, base = rgev, 2
                else:
                    src, base = rgov, 1 + dj
                patch = src[:, SUBR * s + 1 + di:SUBR * s + 5 + di,
                            base:base + 256]
                dkv = dk[:].rearrange("p (r w) -> p r w", w=W)
                prod = prod_p.tile([128, 1024], BF16)
                prodv = prod[:].rearrange("p (r w) -> p r w", w=W)
                if PATHS[ij] == "A":
                    nc.vector.scalar_tensor_tensor(
                        prodv[:], dkv[:], bt[:, 1 + ij:2 + ij], patch[:],
                        op0=AOP.add, op1=AOP.mult)
                else:
                    dkb = dkb_p.tile([128, 1024], BF16)
                    nc.scalar.activation(dkb[:], dk[:], IDENT,
                                         bias=bt[:, 1 + ij:2 + ij], scale=1.0)
                    dkbv = dkb[:].rearrange("p (r w) -> p r w", w=W)
                    eng = nc.gpsimd if PATHS[ij] == "B" else nc.vector
                    eng.tensor_tensor(prodv[:], dkbv[:], patch[:],
                                      op=AOP.mult)
                prods[ij] = prod

            # partial tree-merge of the late (conv-path) prods on DVE/GPSIMD
            t0 = prod_p.tile([128, 1024], BF16, tag="tm")
            nc.vector.tensor_tensor(t0[:], prods[1][:], prods[4][:], op=AOP.add)
            t1 = prod_p.tile([128, 1024], BF16, tag="tm")
            nc.gpsimd.tensor_tensor(t1[:], prods[7][:], prods[8][:], op=AOP.add)
            t2 = prod_p.tile([128, 1024], BF16, tag="tm")
            nc.vector.tensor_tensor(t2[:], t0[:], t1[:], op=AOP.add)

            # ---- reduction: out[:, px] = sum of 6 tiles on the PE via
            # identity-matmul accumulation into two psum chains. First MM of
            # each bank is a full-K=128 identity with start=True (clears the
            # bank's has_written bits); the rest are two concurrent K=64
            # diagonal-block MMs at (0,0)/(64,64) accumulating on top.
            oc0 = po_p.tile([128, 512], F32, tag="oc")
            oc1 = po_p.tile([128, 512], F32, tag="oc")
            tiles = [prods[ij] for ij in IDMM_ORDER] + [t2]
            for k, pt in enumerate(tiles):
                sp = k == len(tiles) - 1
                for nh, oc in ((0, oc0), (1, oc1)):
                    cs = slice(512 * nh, 512 * nh + 512)
                    if k == 0 or sp:
                        # first and last MM of each chain span all 128
                        # partitions so the bank's has_written clear (first)
                        # and the sim's group-close (last) cover the bank.
                        nc.tensor.matmul(oc[:], idt[:], pt[:, cs],
                                         start=(k == 0), stop=sp,
                                         tile_position=(0, 0))
                    else:
                        nc.tensor.matmul(oc[0:64, :], idt[0:64, 0:64],
                                         pt[0:64, cs], start=False, stop=False,
                                         tile_position=(0, 0))
                        nc.tensor.matmul(oc[64:128, :], idt[64:128, 64:128],
                                         pt[64:128, cs], start=False, stop=False,
                                         tile_position=(64, 64))

            ot = outt_p.tile([128, 1024], F32)
            nc.scalar.copy(ot[:, 0:512], oc0[:])
            nc.scalar.copy(ot[:, 512:1024], oc1[:])

            otv = ot[:].rearrange("p (r w) -> p r w", w=W)
            ra = t * RBLK + SUBR * s
            nc.sync.dma_start(out[:, ra:ra + SUBR, :], otv[0:64, :, :])
            nc.sync.dma_start(out[:, HALF + ra:HALF + ra + SUBR, :],
                              otv[64:128, :, :])


def _prep_consts(W1, b1, W2, b2):
    W1T = np.ascontiguousarray(W1.T)                              # [96, 32]
    W1T4 = np.tile(W1T, (1, 4))                                   # [96, 128]
    w1sb = np.zeros((128, 384), np.float32)
    w1sb[0:64, 0:128] = W1T4[0:64]          # rgb A
    w1sb[64:96, 0:128] = W1T4[64:96]        # ev A
    w1sb[64:128, 128:256] = W1T4[0:64]      # rgb B
    w1sb[96:128, 256:384] = W1T4[64:96]     # ev B

    W2r = W2.reshape(C, 9, MID)
    w2sb = np.zeros((128, 384), np.float32)
    for ij in range(9):
        rg, slot = ij % 4, ij // 4
        wij = np.ascontiguousarray(W2r[:, ij, :].T)               # [32, 64]
        w2sb[32 * rg:32 * rg + 32, 128 * slot:128 * slot + 64] = wij
        w2sb[32 * rg:32 * rg + 32, 128 * slot + 64:128 * slot + 128] = wij

    bisb = np.zeros((128, 10), np.float32)
    bisb[:, 0] = np.tile(b1, 4)
    b2r = b2.reshape(C, 9)
    for ij in range(9):
        bisb[:, 1 + ij] = np.concatenate([b2r[:, ij], b2r[:, ij]])
    return w1sb.astype(BF), w2sb.astype(BF), bisb


def _shard_inputs(rgb_feature, event_feature, W1, b1, W2, b2):
    rgbp = np.pad(rgb_feature, ((0, 0), (0, 0), (1, 1), (1, 1)), mode="reflect")
    # two bf16 copies of the padded slab: pixel col c at element c+2 (even
    # view, serves dj=0) and at element c+1 (odd view, serves dj=+-1).
    rgbe = np.zeros((B, C, H + 2, WE), BF)
    rgbo = np.zeros((B, C, H + 2, WE), BF)
    rgbe[:, :, :, 1:1 + W + 2] = rgbp
    rgbo[:, :, :, 0:W + 2] = rgbp
    evb = event_feature.astype(BF)
    w1sb, w2sb, bisb = _prep_consts(W1, b1, W2, b2)
    idsb = np.eye(128, dtype=BF)
    in_maps = []
    for k in range(NCORES):
        b, r0 = k // 2, SHARD_H * (k % 2)
        in_maps.append({
            "rgbe": np.ascontiguousarray(rgbe[b, :, r0:r0 + SHARD_H + 2, :]),
            "rgbo": np.ascontiguousarray(rgbo[b, :, r0:r0 + SHARD_H + 2, :]),
            "ev": np.ascontiguousarray(evb[b, :, r0:r0 + SHARD_H, :]),
            "w1": w1sb, "w2": w2sb, "bi": bisb, "idm": idsb,
        })
    return in_maps


def _run(inputs, trace=False, **trace_kwargs):
    if "nc" not in _cache:
        _cache["nc"] = _build()
    nc = _cache["nc"]
    in_maps = _shard_inputs(
        inputs["rgb_feature"].astype(np.float32),
        inputs["event_feature"].astype(np.float32),
        inputs["W1"].astype(np.float32), inputs["b1"].astype(np.float32),
        inputs["W2"].astype(np.float32), inputs["b2"].astype(np.float32))
    res = run_bass_kernel_spmd(nc, in_maps, list(range(NCORES)),
                               trace=trace, **trace_kwargs)
    full = np.empty((B, C, H, W), np.float32)
    for k in range(NCORES):
        b, r0 = k // 2, SHARD_H * (k % 2)
        full[b, :, r0:r0 + SHARD_H, :] = res.results[k]["out"]
    return full, res


def kernel(**inputs):
    full, _ = _run(inputs, trace=False)
    return full
